# revision 5
# baseline (speedup 1.0000x reference)
"""Trainium2 Bass kernel for iterative Hopfield update.

x <- softmax(x @ P^T) @ P, 3 iterations.
B=4096, N_PATTERNS=8192, N_NEURONS=1024, fp32.

Sharding: data-parallel over batch across 8 cores (512 rows each),
patterns replicated. All matmuls run in float32r (TF32-like, 1 cycle/row
on the PE at free-dim>=256, measured rel err ~1.5e-4).

Device-side layout trick: everything is kept transposed. Each core holds
XT = x_shard^T [1024, 512] in SBUF and computes

  phase 1 (per pattern tile j of 128):  S^T[j] = PT[j] rows @ XT
          [128 pat, 512 batch] accumulated over 8 neuron k-tiles,
          then E[j] = exp(S^T[j])  (no max subtraction needed: scores
          are bounded by ~|x||p| < 40, well within fp32 exp range),
          and a ones-row matmul accumulates column sums (per-batch
          softmax denominators) into a [1, 512] PSUM tile.
  phase 2 (per neuron tile m of 128):   O^T[m] = sum_j P[j]^T-block @ E[j]
          [128 neur, 512 batch] accumulated over 64 pattern tiles,
          then XT_next[m] = O^T[m] * recip (denominator reciprocals
          broadcast to all partitions via a K=1 ones matmul).

XT_next is exactly the layout phase 1 consumes, so iterations chain with
zero on-device transposes. The host pre-transposes x and pre-tiles
patterns into the two block layouts the matmuls consume.
"""

import numpy as np

B, P, N = 4096, 8192, 1024
N_CORES = 8
BLOC = B // N_CORES          # 512 batch rows per core
NJ = P // 128                # 64 pattern tiles
NK = N // 128                # 8 neuron tiles
N_ITER = 3

_cache = {}
_ONES = np.ones((128, 128), dtype=np.float32)


def _build():
    import concourse.bacc as bacc
    import concourse.tile as tile
    from concourse import mybir

    f32 = mybir.dt.float32
    f32r = mybir.dt.float32r
    EXP = mybir.ActivationFunctionType.Exp

    nc = bacc.Bacc("TRN2", target_bir_lowering=False, debug=False)
    xt_d = nc.dram_tensor("xt", [N, BLOC], f32r, kind="ExternalInput").ap()
    ptb_d = nc.dram_tensor("ptb", [NJ, NK, 128, 128], f32r, kind="ExternalInput").ap()
    pb_d = nc.dram_tensor("pb", [NK, NJ, 128, 128], f32r, kind="ExternalInput").ap()
    ones_d = nc.dram_tensor("ones", [128, 128], f32r, kind="ExternalInput").ap()
    ot_d = nc.dram_tensor("ot", [N, BLOC], f32r, kind="ExternalOutput").ap()

    with tile.TileContext(nc) as tc:
        with (
            tc.tile_pool(name="const", bufs=1) as const_pool,
            tc.tile_pool(name="xt", bufs=2) as xt_pool,
            tc.tile_pool(name="e", bufs=1) as e_pool,
            tc.tile_pool(name="pt", bufs=3) as pt_pool,
            tc.tile_pool(name="p2", bufs=3) as p2_pool,
            tc.tile_pool(name="misc", bufs=2) as misc_pool,
            tc.tile_pool(name="s_ps", bufs=2, space="PSUM") as s_ps_pool,
            tc.tile_pool(name="sum_ps", bufs=1, space="PSUM") as sum_ps_pool,
            tc.tile_pool(name="bc_ps", bufs=1, space="PSUM") as bc_ps_pool,
            tc.tile_pool(name="o_ps", bufs=2, space="PSUM") as o_ps_pool,
        ):
            ones_col = const_pool.tile([128, 1], f32r, tag="ones_col")
            nc.sync.dma_start(ones_col[:], ones_d[:, 0:1])
            ones_row = const_pool.tile([1, 128], f32r, tag="ones_row")
            nc.sync.dma_start(ones_row[:], ones_d[0:1, :])

            # initial XT load
            xt_cur = []
            for k in range(NK):
                t = xt_pool.tile([128, BLOC], f32r, tag=f"xt{k}")
                nc.sync.dma_start(t[:], xt_d[128 * k:128 * (k + 1), :])
                xt_cur.append(t)

            for it in range(N_ITER):
                # ---- phase 1: S^T = P @ x^T per pattern tile, exp, sums ----
                e_tiles = []
                sum_ps = sum_ps_pool.tile([1, BLOC], f32, tag="sum")
                for j in range(NJ):
                    pt_t = pt_pool.tile([128, NK * 128], f32r, tag="pt")
                    for k in range(NK):
                        nc.sync.dma_start(
                            pt_t[:, 128 * k:128 * (k + 1)], ptb_d[j, k]
                        )
                    s_ps = s_ps_pool.tile([128, BLOC], f32, tag="s")
                    for k in range(NK):
                        nc.tensor.matmul(
                            s_ps[:],
                            pt_t[:, 128 * k:128 * (k + 1)],
                            xt_cur[k][:],
                            start=(k == 0),
                            stop=(k == NK - 1),
                        )
                    e_t = e_pool.tile([128, BLOC], f32r, tag=f"e{j}")
                    nc.scalar.activation(e_t[:], s_ps[:], EXP)
                    e_tiles.append(e_t)
                    nc.tensor.matmul(
                        sum_ps[:],
                        ones_col[:],
                        e_t[:],
                        start=(j == 0),
                        stop=(j == NJ - 1),
                        skip_group_check=True,
                    )

                # denominators -> reciprocals broadcast to 128 partitions
                sum_sb = misc_pool.tile([1, BLOC], f32r, tag="sum_sb")
                nc.vector.tensor_copy(sum_sb[:], sum_ps[:])
                bc_ps = bc_ps_pool.tile([128, BLOC], f32, tag="bc")
                nc.tensor.matmul(
                    bc_ps[:],
                    ones_row[:],
                    sum_sb[:],
                    start=True,
                    stop=True,
                )
                recip = misc_pool.tile([128, BLOC], f32, tag="recip")
                nc.vector.reciprocal(recip[:], bc_ps[:])

                # ---- phase 2: O^T = sum_j P_block^T @ E[j], scale, next XT ----
                xt_next = []
                for m in range(NK):
                    o_ps = o_ps_pool.tile([128, BLOC], f32, tag="o")
                    for kc in range(NJ // 4):
                        p2_t = p2_pool.tile([128, 4 * 128], f32r, tag="p2")
                        for g in range(4):
                            nc.sync.dma_start(
                                p2_t[:, 128 * g:128 * (g + 1)],
                                pb_d[m, 4 * kc + g],
                            )
                        for g in range(4):
                            kk = 4 * kc + g
                            nc.tensor.matmul(
                                o_ps[:],
                                p2_t[:, 128 * g:128 * (g + 1)],
                                e_tiles[kk][:],
                                start=(kk == 0),
                                stop=(kk == NJ - 1),
                            )
                    xt_n = xt_pool.tile([128, BLOC], f32r, tag=f"xt{m}")
                    nc.vector.tensor_mul(xt_n[:], o_ps[:], recip[:])
                    xt_next.append(xt_n)
                    if it == N_ITER - 1:
                        nc.sync.dma_start(ot_d[128 * m:128 * (m + 1), :], xt_n[:])
                xt_cur = xt_next

    nc.compile()
    return nc


def kernel(x: np.ndarray, patterns: np.ndarray) -> np.ndarray:
    from concourse.bass_utils import run_bass_kernel_spmd

    if "nc" not in _cache:
        _cache["nc"] = _build()
    nc = _cache["nc"]

    x = np.ascontiguousarray(x, dtype=np.float32)
    patterns = np.ascontiguousarray(patterns, dtype=np.float32)

    # host-side tiling of the replicated patterns
    p4 = patterns.reshape(NJ, 128, NK, 128)          # [j, p, k, n]
    ptb = np.ascontiguousarray(p4.transpose(0, 2, 3, 1))  # [j, k, n, p] (block^T)
    pb = np.ascontiguousarray(p4.transpose(2, 0, 1, 3))   # [m, j, p, n] (natural)
    xt = np.ascontiguousarray(x.T)                   # [N, B]

    in_maps = [
        {
            "xt": np.ascontiguousarray(xt[:, BLOC * i:BLOC * (i + 1)]),
            "ptb": ptb,
            "pb": pb,
            "ones": _ONES,
        }
        for i in range(N_CORES)
    ]
    res = run_bass_kernel_spmd(nc, in_maps, list(range(N_CORES))).results
    out = np.concatenate([res[i]["ot"].T for i in range(N_CORES)], axis=0)
    return np.ascontiguousarray(out.astype(np.float32))


# revision 10
# speedup vs baseline: 100.1578x; 100.1578x over previous
"""Trainium2 Bass kernel for iterative Hopfield update.

x <- softmax(x @ P^T) @ P, 3 iterations.
B=4096, N_PATTERNS=8192, N_NEURONS=1024, fp32.

Sharding: data-parallel over batch across 8 cores (512 rows each),
patterns replicated. All matmuls run in float32r (TF32-like, 1 cycle/row
on the PE at free-dim>=256, measured rel err ~1.5e-4).

Device-side layout trick: everything is kept transposed. Each core holds
XT = x_shard^T [1024, 512] in SBUF and computes

  phase 1 (per pattern tile j of 128):  S^T[j] = PT[j] rows @ XT
          [128 pat, 512 batch] accumulated over 8 neuron k-tiles,
          then E[j] = exp(S^T[j])  (no max subtraction needed: scores
          are bounded by ~|x||p| < 40, well within fp32 exp range).
          The DVE accumulates E tiles elementwise in fp32; one ones-column
          matmul then reduces across partitions for the per-batch softmax
          denominators (PE cost: 1 matmul instead of 64).
  phase 2 (per neuron tile m of 128):   O^T[m] = sum_j P[j]^T-block @ E[j]
          [128 neur, 512 batch] accumulated over 64 pattern tiles,
          then XT_next[m] = O^T[m] * recip (denominator reciprocals
          broadcast to all partitions via a K=1 ones matmul).

XT_next is exactly the layout phase 1 consumes, so iterations chain with
zero on-device transposes. The host pre-transposes x and pre-tiles
patterns into two block layouts chosen so every device DMA is a dense 2D
copy with 4KB contiguous partition lines (one 512KB transfer per 8-subtile
block; small strided DMAs previously made the DMA front-end the
bottleneck at 2.6x the runtime).
"""

import numpy as np

B, P, N = 4096, 8192, 1024
N_CORES = 8
BLOC = B // N_CORES          # 512 batch rows per core
NJ = P // 128                # 64 pattern tiles
NK = N // 128                # 8 neuron tiles
N_ITER = 3
LOOP_REPS = 1   # >1: wrap body in a hardware loop (timing only, wrong numerics)

_cache = {}
_ONES = np.ones((128, 128), dtype=np.float32)


def _build():
    import concourse.bacc as bacc
    import concourse.tile as tile
    from concourse import mybir

    f32 = mybir.dt.float32
    f32r = mybir.dt.float32r
    EXP = mybir.ActivationFunctionType.Exp

    nc = bacc.Bacc("TRN2", target_bir_lowering=False, debug=False)
    xt_d = nc.dram_tensor("xt", [N, BLOC], f32r, kind="ExternalInput").ap()
    ptb_d = nc.dram_tensor("ptb", [NJ, 128, NK * 128], f32r, kind="ExternalInput").ap()
    pb_d = nc.dram_tensor("pb", [NK, NJ // 8, 128, 8 * 128], f32r, kind="ExternalInput").ap()
    ones_d = nc.dram_tensor("ones", [128, 128], f32r, kind="ExternalInput").ap()
    ot_d = nc.dram_tensor("ot", [N, BLOC], f32r, kind="ExternalOutput").ap()

    with tile.TileContext(nc) as tc:
        with (
            tc.tile_pool(name="const", bufs=1) as const_pool,
            tc.tile_pool(name="xt", bufs=2) as xt_pool,
            tc.tile_pool(name="e", bufs=1) as e_pool,
            tc.tile_pool(name="pt", bufs=4) as pt_pool,
            tc.tile_pool(name="p2", bufs=4) as p2_pool,
            tc.tile_pool(name="misc", bufs=1) as misc_pool,
            tc.tile_pool(name="s_ps", bufs=4, space="PSUM") as s_ps_pool,
            tc.tile_pool(name="sum_ps", bufs=1, space="PSUM") as sum_ps_pool,
            tc.tile_pool(name="bc_ps", bufs=1, space="PSUM") as bc_ps_pool,
            tc.tile_pool(name="o_ps", bufs=2, space="PSUM") as o_ps_pool,
        ):
            ones_col = const_pool.tile([128, 1], f32r, tag="ones_col")
            nc.sync.dma_start(ones_col[:], ones_d[:, 0:1])
            ones_row = const_pool.tile([1, 128], f32r, tag="ones_row")
            nc.sync.dma_start(ones_row[:], ones_d[0:1, :])

            # initial XT load
            xt_cur = []
            for k in range(NK):
                t = xt_pool.tile([128, BLOC], f32r, tag=f"xt{k}")
                nc.sync.dma_start(t[:], xt_d[128 * k:128 * (k + 1), :])
                xt_cur.append(t)

            import contextlib
            loop_cm = (tc.For_i(0, LOOP_REPS) if LOOP_REPS > 1
                       else contextlib.nullcontext())
            with loop_cm:
              for it in range(N_ITER):
                  # ---- phase 1: S^T = P @ x^T per pattern tile, exp, sums ----
                  e_tiles = []
                  acc = misc_pool.tile([128, BLOC], f32, tag="acc")
                  for j in range(NJ):
                      pt_t = pt_pool.tile([128, NK * 128], f32r, tag="pt")
                      nc.sync.dma_start(pt_t[:], ptb_d[j])
                      s_ps = s_ps_pool.tile([128, BLOC], f32, tag="s")
                      for k in range(NK):
                          nc.tensor.matmul(
                              s_ps[:],
                              pt_t[:, 128 * k:128 * (k + 1)],
                              xt_cur[k][:],
                              start=(k == 0),
                              stop=(k == NK - 1),
                          )
                      e_t = e_pool.tile([128, BLOC], f32r, tag=f"e{j}")
                      nc.scalar.activation(e_t[:], s_ps[:], EXP)
                      e_tiles.append(e_t)
                      # softmax denominators: accumulate E on DVE (PE stays on matmuls)
                      if j == 0:
                          nc.vector.tensor_copy(acc[:], e_t[:])
                      else:
                          nc.vector.tensor_add(acc[:], acc[:], e_t[:])

                  # cross-partition reduce via one ones-matmul: sum_ps[0, b] = sum_p acc[p, b]
                  acc_r = misc_pool.tile([128, BLOC], f32r, tag="acc_r")
                  nc.vector.tensor_copy(acc_r[:], acc[:])
                  sum_ps = sum_ps_pool.tile([1, BLOC], f32, tag="sum")
                  nc.tensor.matmul(sum_ps[:], ones_col[:], acc_r[:], start=True, stop=True)

                  # denominators -> reciprocals broadcast to 128 partitions
                  sum_sb = misc_pool.tile([1, BLOC], f32r, tag="sum_sb")
                  nc.vector.tensor_copy(sum_sb[:], sum_ps[:])
                  bc_ps = bc_ps_pool.tile([128, BLOC], f32, tag="bc")
                  nc.tensor.matmul(
                      bc_ps[:],
                      ones_row[:],
                      sum_sb[:],
                      start=True,
                      stop=True,
                  )
                  recip = misc_pool.tile([128, BLOC], f32, tag="recip")
                  nc.vector.reciprocal(recip[:], bc_ps[:])

                  # ---- phase 2: O^T = sum_j P_block^T @ E[j], scale, next XT ----
                  xt_next = []
                  for m in range(NK):
                      o_ps = o_ps_pool.tile([128, BLOC], f32, tag="o")
                      for kc in range(NJ // 8):
                          p2_t = p2_pool.tile([128, 8 * 128], f32r, tag="p2")
                          nc.sync.dma_start(p2_t[:], pb_d[m, kc])
                          for g in range(8):
                              kk = 8 * kc + g
                              nc.tensor.matmul(
                                  o_ps[:],
                                  p2_t[:, 128 * g:128 * (g + 1)],
                                  e_tiles[kk][:],
                                  start=(kk == 0),
                                  stop=(kk == NJ - 1),
                              )
                      xt_n = xt_pool.tile([128, BLOC], f32r, tag=f"xt{m}")
                      nc.vector.tensor_mul(xt_n[:], o_ps[:], recip[:])
                      xt_next.append(xt_n)
                      if it == N_ITER - 1:
                          nc.sync.dma_start(ot_d[128 * m:128 * (m + 1), :], xt_n[:])
                  xt_cur = xt_next

    nc.compile()
    return nc


def _prepare_inputs(x: np.ndarray, patterns: np.ndarray) -> list:
    x = np.ascontiguousarray(x, dtype=np.float32)
    patterns = np.ascontiguousarray(patterns, dtype=np.float32)

    # host-side tiling of the replicated patterns
    p4 = patterns.reshape(NJ, 128, NK, 128)          # [j, p, k, n]
    # ptb[j, n, k*128+p]: SBUF partition line n of block j, k-subtiles contiguous
    ptb = np.ascontiguousarray(p4.transpose(0, 3, 2, 1)).reshape(NJ, 128, NK * 128)
    # pb[m, kc, pat, g*128+n]: partition line pat, 8 k-subtiles contiguous
    pb = np.ascontiguousarray(
        p4.transpose(2, 0, 1, 3).reshape(NK, NJ // 8, 8, 128, 128)
          .transpose(0, 1, 3, 2, 4)
    ).reshape(NK, NJ // 8, 128, 8 * 128)
    xt = np.ascontiguousarray(x.T)                   # [N, B]
    return [
        {
            "xt": np.ascontiguousarray(xt[:, BLOC * i:BLOC * (i + 1)]),
            "ptb": ptb,
            "pb": pb,
            "ones": _ONES,
        }
        for i in range(N_CORES)
    ]


def kernel(x: np.ndarray, patterns: np.ndarray) -> np.ndarray:
    from concourse.bass_utils import run_bass_kernel_spmd

    if "nc" not in _cache:
        _cache["nc"] = _build()
    nc = _cache["nc"]

    in_maps = _prepare_inputs(x, patterns)
    res = run_bass_kernel_spmd(nc, in_maps, list(range(N_CORES))).results
    out = np.concatenate([res[i]["ot"].T for i in range(N_CORES)], axis=0)
    return np.ascontiguousarray(out.astype(np.float32))

